# revision 13
# baseline (speedup 1.0000x reference)
"""PathRNN Trainium2 kernel.

Math (per reference):
  x_proj = vel @ W_ih.T + b_ih + b_hh            # [B, T, H]
  h_t    = tanh(x_proj[:, t] + h_{t-1} @ W_hh.T) # scan over T, h_0 = 0
  u      = stack(h_t)                            # [B, T, H]
  pos    = u @ W_out.T + b_out                   # [B, T, 2]

Sharding: data-parallel over batch. B=128 -> 16 per core x 8 cores.
Each core runs the full sequential scan on its batch shard; weights
replicated.

On-core layout: hidden state kept H-on-partitions as 4 chunks of
[128, BS]. Recurrence step = 16 matmuls (4 i-chunks x 4 j-chunks) of
W_hh tiles [128,128] accumulating into PSUM, on top of a per-block
prepass matmul that puts x_proj into the same PSUM columns. Tanh on
the scalar engine (bias = b_ih+b_hh per partition) writes the u tile,
which is both the next step's matmul rhs and the DMA-out buffer.
"""

import numpy as np

import concourse.bass as bass
import concourse.tile as tile
from concourse.bass import mybir
from concourse.bass_utils import run_bass_kernel_spmd
import bass_rust as _bass_rust

B, T, H = 128, 1024, 512
NCORES = 8
BS = B // NCORES          # batch per core
CH = H // 128             # H chunks of 128
BLK = 16                  # timesteps per block
F32 = mybir.dt.float32


def build_bass(seq_len=T, blk=BLK):
    nblk = seq_len // blk
    nc = bass.Bass("TRN2", target_bir_lowering=False, debug=False,
                   num_devices=NCORES)

    velT_d = nc.dram_tensor("velT", [2, seq_len, BS], F32, kind="ExternalInput")
    whhT_d = nc.dram_tensor("whhT", [128, CH * CH, 128], F32, kind="ExternalInput")
    wihT_d = nc.dram_tensor("wihT", [2, H], F32, kind="ExternalInput")
    woutT_d = nc.dram_tensor("woutT", [128, CH, 2], F32, kind="ExternalInput")
    bias_d = nc.dram_tensor("bias", [128, CH], F32, kind="ExternalInput")
    bout_d = nc.dram_tensor("bout", [2, 1], F32, kind="ExternalInput")

    u_d = nc.dram_tensor("u", [128, CH, seq_len, BS], F32, kind="ExternalOutput")
    pos_d = nc.dram_tensor("pos", [2, seq_len, BS], F32, kind="ExternalOutput")

    with tile.TileContext(nc) as tc:
        with (
            tc.tile_pool(name="wpool", bufs=1) as wpool,
            tc.tile_pool(name="vpool", bufs=2) as vpool,
            tc.tile_pool(name="upool", bufs=2) as upool,
            tc.tile_pool(name="ppool", bufs=2) as ppool,
            tc.tile_pool(name="zpool", bufs=2, space="PSUM") as zpool,
            tc.tile_pool(name="pppool", bufs=2, space="PSUM") as pppool,
        ):
            whhT = wpool.tile([128, CH * CH, 128], F32)
            wihT = wpool.tile([2, H], F32)
            woutT = wpool.tile([128, CH, 2], F32)
            bias = wpool.tile([128, CH], F32)
            bout = wpool.tile([2, 1], F32)
            hinit = wpool.tile([128, CH, BS], F32)

            nc.sync.dma_start(whhT[:], whhT_d[:])
            nc.sync.dma_start(wihT[:], wihT_d[:])
            nc.sync.dma_start(woutT[:], woutT_d[:])
            nc.sync.dma_start(bias[:], bias_d[:])
            nc.sync.dma_start(bout[:], bout_d[:])
            nc.vector.memset(hinit[:], 0.0)

            # h_{t-1} access: (tile, time-index or None for hinit)
            prev = None  # None -> hinit

            for b in range(nblk):
                t0 = b * blk
                velT = vpool.tile([2, blk, BS], F32, name="velT_sb")
                nc.sync.dma_start(velT[:], velT_d[:, t0:t0 + blk, :])

                z = zpool.tile([128, CH, blk, BS], F32, name="z")
                u = upool.tile([128, CH, blk, BS], F32, name="u_sb")

                # PSUM accumulation groups are per 2KB bank (zero region):
                # start=True lazily zeroes the WHOLE bank and must come only
                # on the first matmul touching it; stop=True closes the
                # whole bank and must come only on the last. z spans 2 banks
                # (i in {0,1} and {2,3}).

                # x_proj prepass for the whole block
                for i in range(CH):
                    nc.tensor.matmul(
                        z[:, i, :, :], wihT[:, i * 128:(i + 1) * 128], velT[:],
                        start=(i % 2 == 0), stop=False,
                    )

                for tl in range(blk):
                    for i in range(CH):
                        for j in range(CH):
                            if prev is None:
                                rhs = hinit[:, j, :]
                            else:
                                pu, ptl = prev
                                rhs = pu[:, j, ptl, :]
                            nc.tensor.matmul(
                                z[:, i, tl, :], whhT[:, j * CH + i, :], rhs,
                                start=False,
                                stop=(tl == blk - 1 and i % 2 == 1
                                      and j == CH - 1),
                            )
                        nc.scalar.activation(
                            u[:, i, tl, :], z[:, i, tl, :],
                            mybir.ActivationFunctionType.Tanh,
                            bias=bias[:, i:i + 1],
                        )
                    prev = (u, tl)

                pp = pppool.tile([2, blk, BS], F32, name="pp")
                for j in range(CH):
                    nc.tensor.matmul(
                        pp[:], woutT[:, j, :], u[:, j, :, :],
                        start=(j == 0), stop=(j == CH - 1),
                    )
                pos_sb = ppool.tile([2, blk, BS], F32, name="pos_sb")
                nc.vector.tensor_scalar_add(pos_sb[:], pp[:], bout[:, 0:1])

                nc.sync.dma_start(u_d[:, :, t0:t0 + blk, :], u[:])
                nc.sync.dma_start(pos_d[:, t0:t0 + blk, :], pos_sb[:])

    # Walrus codegen allows at most 1 sync wait per instruction (Matmult
    # LDWEIGHTS especially); this Bacc pass splits excess waits into
    # InstEventSemaphore pairs.
    _bass_rust.generate_event_semaphores(nc)
    return nc


def pack_inputs(vel, W_ih, W_hh, b_ih, b_hh, W_out, b_out, seq_len=T):
    vel = np.asarray(vel, dtype=np.float32)
    W_ih = np.asarray(W_ih, dtype=np.float32)
    W_hh = np.asarray(W_hh, dtype=np.float32)
    W_out = np.asarray(W_out, dtype=np.float32)
    bsum = (np.asarray(b_ih, dtype=np.float32)
            + np.asarray(b_hh, dtype=np.float32))

    # velT[c, t, b] = vel[b, t, c]
    velT = np.ascontiguousarray(vel.transpose(2, 1, 0))
    # whhT[k1, j*CH+i, m] = W_hh[i*128+m, j*128+k1]
    whhT = np.ascontiguousarray(
        W_hh.reshape(CH, 128, CH, 128).transpose(3, 2, 0, 1)
        .reshape(128, CH * CH, 128))
    wihT = np.ascontiguousarray(W_ih.T)                      # [2, H]
    # woutT[k1, j, o] = W_out[o, j*128+k1]
    woutT = np.ascontiguousarray(
        W_out.T.reshape(CH, 128, 2).transpose(1, 0, 2))       # [128, CH, 2]
    bias = np.ascontiguousarray(bsum.reshape(CH, 128).T)      # [128, CH]
    bout = np.ascontiguousarray(
        np.asarray(b_out, dtype=np.float32).reshape(2, 1))

    in_maps = []
    for c in range(NCORES):
        in_maps.append({
            "velT": np.ascontiguousarray(velT[:, :, c * BS:(c + 1) * BS]),
            "whhT": whhT,
            "wihT": wihT,
            "woutT": woutT,
            "bias": bias,
            "bout": bout,
        })
    return in_maps


def unpack_outputs(results, seq_len=T):
    pos_parts, u_parts = [], []
    for r in results:
        u_d = r["u"]          # [128, CH, seq_len, BS]
        pos_d = r["pos"]      # [2, seq_len, BS]
        u_parts.append(np.ascontiguousarray(
            u_d.transpose(3, 2, 1, 0).reshape(BS, seq_len, H)))
        pos_parts.append(np.ascontiguousarray(pos_d.transpose(2, 1, 0)))
    u_full = np.concatenate(u_parts, axis=0).astype(np.float32, copy=False)
    pos_full = np.concatenate(pos_parts, axis=0).astype(np.float32, copy=False)
    return pos_full, u_full


_NC_CACHE = {}


def run(vel, W_ih, W_hh, b_ih, b_hh, W_out, b_out, seq_len=T, trace=False,
        blk=BLK):
    key = (seq_len, blk)
    if key not in _NC_CACHE:
        _NC_CACHE[key] = build_bass(seq_len=seq_len, blk=blk)
    nc = _NC_CACHE[key]
    in_maps = pack_inputs(vel, W_ih, W_hh, b_ih, b_hh, W_out, b_out,
                          seq_len=seq_len)
    res = run_bass_kernel_spmd(nc, in_maps, core_ids=list(range(NCORES)),
                               trace=trace)
    pos_full, u_full = unpack_outputs(res.results, seq_len=seq_len)
    return (pos_full, u_full), res


def kernel(vel, W_ih, W_hh, b_ih, b_hh, W_out, b_out):
    (pos_full, u_full), _ = run(vel, W_ih, W_hh, b_ih, b_hh, W_out, b_out)
    return pos_full, u_full


# revision 14
# speedup vs baseline: 7.7103x; 7.7103x over previous
"""PathRNN Trainium2 kernel.

Math (per reference):
  x_proj = vel @ W_ih.T + b_ih + b_hh            # [B, T, H]
  h_t    = tanh(x_proj[:, t] + h_{t-1} @ W_hh.T) # scan over T, h_0 = 0
  u      = stack(h_t)                            # [B, T, H]
  pos    = u @ W_out.T + b_out                   # [B, T, 2]

Sharding: data-parallel over batch. B=128 -> 16 per core x 8 cores.
Each core runs the full sequential scan on its batch shard; weights
replicated.

On-core layout: hidden state kept H-on-partitions as 4 chunks of
[128, BS]. Recurrence step = 16 fp16 matmuls (4 i-chunks x 4 j-chunks)
of W_hh tiles [128,128] accumulating into fp32 PSUM, on top of a
per-block prepass matmul that puts x_proj (+ bias, via a ones-row in
the vel input so K=3) into the same PSUM columns. One tanh per step on
the scalar engine covering all 4 chunks writes the fp16 u tile, which
is both the next step's matmul rhs and the DMA-out buffer (host
upcasts u to fp32).
"""

import numpy as np

import concourse.bass as bass
import concourse.tile as tile
from concourse.bass import mybir
from concourse.bass_utils import run_bass_kernel_spmd
import bass_rust as _bass_rust

B, T, H = 128, 1024, 512
NCORES = 8
BS = B // NCORES          # batch per core
CH = H // 128             # H chunks of 128
BLK = 16                  # timesteps per block
F32 = mybir.dt.float32
F16 = mybir.dt.float16


def build_bass(seq_len=T, blk=BLK):
    nblk = seq_len // blk
    nc = bass.Bass("TRN2", target_bir_lowering=False, debug=False,
                   num_devices=NCORES)

    velT_d = nc.dram_tensor("velT", [3, seq_len, BS], F16, kind="ExternalInput")
    whhT_d = nc.dram_tensor("whhT", [128, CH * CH, 128], F16, kind="ExternalInput")
    wihT_d = nc.dram_tensor("wihT", [3, H], F16, kind="ExternalInput")
    woutT_d = nc.dram_tensor("woutT", [128, CH, 2], F16, kind="ExternalInput")
    bout_d = nc.dram_tensor("bout", [2, 1], F32, kind="ExternalInput")

    u_d = nc.dram_tensor("u", [128, CH, seq_len, BS], F16, kind="ExternalOutput")
    pos_d = nc.dram_tensor("pos", [2, seq_len, BS], F32, kind="ExternalOutput")

    with tile.TileContext(nc) as tc:
        with (
            tc.tile_pool(name="wpool", bufs=1) as wpool,
            tc.tile_pool(name="vpool", bufs=2) as vpool,
            tc.tile_pool(name="upool", bufs=2) as upool,
            tc.tile_pool(name="ppool", bufs=2) as ppool,
            tc.tile_pool(name="zpool", bufs=2, space="PSUM") as zpool,
            tc.tile_pool(name="pppool", bufs=2, space="PSUM") as pppool,
        ):
            whhT = wpool.tile([128, CH * CH, 128], F16)
            wihT = wpool.tile([3, H], F16)
            woutT = wpool.tile([128, CH, 2], F16)
            bout = wpool.tile([2, 1], F32)
            hinit = wpool.tile([128, CH, BS], F16)

            nc.sync.dma_start(whhT[:], whhT_d[:])
            nc.sync.dma_start(wihT[:], wihT_d[:])
            nc.sync.dma_start(woutT[:], woutT_d[:])
            nc.sync.dma_start(bout[:], bout_d[:])
            nc.vector.memset(hinit[:], 0.0)

            # h_{t-1} access: (tile, time-index or None for hinit)
            prev = None  # None -> hinit

            for b in range(nblk):
                t0 = b * blk
                velT = vpool.tile([3, blk, BS], F16, name="velT_sb")
                nc.sync.dma_start(velT[:], velT_d[:, t0:t0 + blk, :])

                z = zpool.tile([128, CH, blk, BS], F32, name="z")
                u = upool.tile([128, CH, blk, BS], F16, name="u_sb")

                # PSUM accumulation groups are per 2KB bank (zero region):
                # start=True lazily zeroes the WHOLE bank and must come only
                # on the first matmul touching it; stop=True closes the
                # whole bank and must come only on the last. z spans 2 banks
                # (i in {0,1} and {2,3}).

                # x_proj (+ bias via ones-row) prepass for the whole block
                for i in range(CH):
                    nc.tensor.matmul(
                        z[:, i, :, :], wihT[:, i * 128:(i + 1) * 128], velT[:],
                        start=(i % 2 == 0), stop=False,
                    )

                for tl in range(blk):
                    for i in range(CH):
                        for j in range(CH):
                            if prev is None:
                                rhs = hinit[:, j, :]
                            else:
                                pu, ptl = prev
                                rhs = pu[:, j, ptl, :]
                            nc.tensor.matmul(
                                z[:, i, tl, :], whhT[:, j * CH + i, :], rhs,
                                start=False,
                                stop=(tl == blk - 1 and i % 2 == 1
                                      and j == CH - 1),
                            )
                    nc.scalar.activation(
                        u[:, :, tl, :], z[:, :, tl, :],
                        mybir.ActivationFunctionType.Tanh,
                    )
                    prev = (u, tl)

                pp = pppool.tile([2, blk, BS], F32, name="pp")
                for j in range(CH):
                    nc.tensor.matmul(
                        pp[:], woutT[:, j, :], u[:, j, :, :],
                        start=(j == 0), stop=(j == CH - 1),
                    )
                pos_sb = ppool.tile([2, blk, BS], F32, name="pos_sb")
                nc.vector.tensor_scalar_add(pos_sb[:], pp[:], bout[:, 0:1])

                nc.sync.dma_start(u_d[:, :, t0:t0 + blk, :], u[:])
                nc.sync.dma_start(pos_d[:, t0:t0 + blk, :], pos_sb[:])

    # Walrus codegen allows at most 1 sync wait per instruction (Matmult
    # LDWEIGHTS especially); this Bacc pass splits excess waits into
    # InstEventSemaphore pairs.
    _bass_rust.generate_event_semaphores(nc)
    return nc


def pack_inputs(vel, W_ih, W_hh, b_ih, b_hh, W_out, b_out, seq_len=T):
    vel = np.asarray(vel, dtype=np.float32)
    W_ih = np.asarray(W_ih, dtype=np.float32)
    W_hh = np.asarray(W_hh, dtype=np.float32)
    W_out = np.asarray(W_out, dtype=np.float32)
    bsum = (np.asarray(b_ih, dtype=np.float32)
            + np.asarray(b_hh, dtype=np.float32))

    # velT[c, t, b] = vel[b, t, c]; row 2 = ones (bias via prepass matmul)
    velT = np.concatenate(
        [vel.transpose(2, 1, 0), np.ones((1, seq_len, B), np.float32)],
        axis=0).astype(np.float16)
    # whhT[k1, j*CH+i, m] = W_hh[i*128+m, j*128+k1]
    whhT = np.ascontiguousarray(
        W_hh.reshape(CH, 128, CH, 128).transpose(3, 2, 0, 1)
        .reshape(128, CH * CH, 128)).astype(np.float16)
    wihT = np.concatenate([W_ih.T, bsum[None, :]], axis=0).astype(np.float16)
    # woutT[k1, j, o] = W_out[o, j*128+k1]
    woutT = np.ascontiguousarray(
        W_out.T.reshape(CH, 128, 2).transpose(1, 0, 2)).astype(np.float16)
    bout = np.ascontiguousarray(
        np.asarray(b_out, dtype=np.float32).reshape(2, 1))

    in_maps = []
    for c in range(NCORES):
        in_maps.append({
            "velT": np.ascontiguousarray(velT[:, :, c * BS:(c + 1) * BS]),
            "whhT": whhT,
            "wihT": wihT,
            "woutT": woutT,
            "bout": bout,
        })
    return in_maps


def unpack_outputs(results, seq_len=T):
    pos_parts, u_parts = [], []
    for r in results:
        u_d = r["u"]          # [128, CH, seq_len, BS] fp16
        pos_d = r["pos"]      # [2, seq_len, BS] fp32
        u_parts.append(np.ascontiguousarray(
            u_d.astype(np.float32).transpose(3, 2, 1, 0)
            .reshape(BS, seq_len, H)))
        pos_parts.append(np.ascontiguousarray(pos_d.transpose(2, 1, 0)))
    u_full = np.concatenate(u_parts, axis=0).astype(np.float32, copy=False)
    pos_full = np.concatenate(pos_parts, axis=0).astype(np.float32, copy=False)
    return pos_full, u_full


_NC_CACHE = {}


def run(vel, W_ih, W_hh, b_ih, b_hh, W_out, b_out, seq_len=T, trace=False,
        blk=BLK):
    key = (seq_len, blk)
    if key not in _NC_CACHE:
        _NC_CACHE[key] = build_bass(seq_len=seq_len, blk=blk)
    nc = _NC_CACHE[key]
    in_maps = pack_inputs(vel, W_ih, W_hh, b_ih, b_hh, W_out, b_out,
                          seq_len=seq_len)
    res = run_bass_kernel_spmd(nc, in_maps, core_ids=list(range(NCORES)),
                               trace=trace)
    pos_full, u_full = unpack_outputs(res.results, seq_len=seq_len)
    return (pos_full, u_full), res


def kernel(vel, W_ih, W_hh, b_ih, b_hh, W_out, b_out):
    (pos_full, u_full), _ = run(vel, W_ih, W_hh, b_ih, b_hh, W_out, b_out)
    return pos_full, u_full
